# revision 1
# baseline (speedup 1.0000x reference)
"""Trainium2 Bass kernel for nn_ExpandingLinear.

Computation (see reference):
    x_exp = concat([x, x[:, p0] * v0, x_exp1[:, p1] * v1], axis=1)   # [B, 2176]
    W     = scatter_add(weight_vals at [weight_rows, weight_cols])    # [2048, 2176]
    b     = scatter_add(bias_vals at bias_idx)                        # [2048]
    out   = x_exp @ W.T + b                                           # [B, 2048]

Sharding: data-parallel over the batch dim across 8 NeuronCores (1024 rows
per core); the weight/bias/embed parameters are replicated.

Host-side prep is limited to sharding and parameter/layout preparation
(batch split, x transpose, COO->dense weight densification, embed parent-chain
resolution); all O(batch) compute — the embed feature construction, the
full dense matmul and the bias add — runs on device.

Device kernel (per core):
  - xt [2048, 1024] (x shard, feature-major) streamed in as [128,128] k-tiles
  - wt [2176, 2048] (W^T) resident in SBUF
  - 128 embed features built on device: indirect-DMA row gather from xt
    + per-partition scale; forms contraction k-tile 16
  - out[m*128:(m+1)*128, :] = sum_k xt_tile[k,m].T @ wt_tile[k] + bias
    (PE matmul in float32r, PSUM fp32 accumulation over 17 k-tiles)
"""

import numpy as np
from contextlib import ExitStack

OUT = 2048
IN_BASE = 2048
N_EMBED = 64
IN_TOT = IN_BASE + 2 * N_EMBED  # 2176
BATCH = 8192
N_CORES = 8
B_CORE = BATCH // N_CORES       # 1024
P = 128
K_TILES = IN_TOT // P           # 17
M_TILES = B_CORE // P           # 8
N_SPLIT = 4                     # 2048 out cols in 4 x 512 (one PSUM bank each)

_CACHED = {}


def _build_nc():
    import concourse.bass as bass
    import concourse.mybir as mybir
    import concourse.tile as tile
    from concourse import bacc

    f32 = mybir.dt.float32
    f32r = mybir.dt.float32r
    i32 = mybir.dt.int32

    nc = bacc.Bacc("TRN2", target_bir_lowering=False, debug=False,
                   num_devices=N_CORES)

    xt = nc.dram_tensor("xt", [IN_BASE, B_CORE], f32r, kind="ExternalInput")
    wt = nc.dram_tensor("wt", [IN_TOT, OUT], f32r, kind="ExternalInput")
    bias = nc.dram_tensor("bias", [P, OUT], f32, kind="ExternalInput")
    emb_q = nc.dram_tensor("emb_q", [P, 1], i32, kind="ExternalInput")
    emb_a = nc.dram_tensor("emb_a", [P, 1], f32, kind="ExternalInput")
    out = nc.dram_tensor("out", [B_CORE, OUT], f32, kind="ExternalOutput")

    with tile.TileContext(nc) as tc:
        with ExitStack() as ctx:
            wt_pool = ctx.enter_context(tc.tile_pool(name="wt", bufs=K_TILES))
            xt_pool = ctx.enter_context(tc.tile_pool(name="xt", bufs=2))
            small_pool = ctx.enter_context(tc.tile_pool(name="small", bufs=1))
            out_pool = ctx.enter_context(tc.tile_pool(name="out", bufs=2))
            psum_pool = ctx.enter_context(
                tc.tile_pool(name="psum", bufs=2, space="PSUM"))

            # W^T resident in SBUF: 17 k-tiles of [128, 2048]
            wt_ap3 = wt.ap().rearrange("(k p) n -> p k n", p=P)  # [128,17,2048]
            wt_tiles = []
            for k in range(K_TILES):
                wtile = wt_pool.tile([P, OUT], f32r, tag="wt")
                nc.sync.dma_start(out=wtile[:], in_=wt_ap3[:, k, :])
                wt_tiles.append(wtile)

            # bias, embed params
            bias_t = small_pool.tile([P, OUT], f32, tag="bias")
            nc.sync.dma_start(out=bias_t[:], in_=bias.ap())
            q_t = small_pool.tile([P, 1], i32, tag="q")
            nc.sync.dma_start(out=q_t[:], in_=emb_q.ap())
            a_t = small_pool.tile([P, 1], f32, tag="a")
            nc.sync.dma_start(out=a_t[:], in_=emb_a.ap())

            # embed features: gather parent rows of xt, scale by alpha.
            # partition j = expanded feature 2048+j; k-tile 16 of x_exp^T.
            emb_raw = small_pool.tile([P, B_CORE], f32r, tag="emb_raw")
            nc.gpsimd.indirect_dma_start(
                out=emb_raw[:],
                out_offset=None,
                in_=xt.ap(),
                in_offset=bass.IndirectOffsetOnAxis(ap=q_t[:, 0:1], axis=0),
            )
            emb_t = small_pool.tile([P, B_CORE], f32r, tag="emb")
            nc.vector.tensor_scalar_mul(
                emb_t[:], emb_raw[:].bitcast(f32), a_t[:, 0:1])

            xt_ap3 = xt.ap().rearrange("(k p) b -> p k b", p=P)  # [128,16,1024]

            for m in range(M_TILES):
                xt_m = xt_pool.tile([P, (K_TILES - 1) * P], f32r, tag="xt")
                nc.sync.dma_start(
                    out=xt_m[:].rearrange("p (k f) -> p k f", k=K_TILES - 1),
                    in_=xt_ap3[:, :, m * P:(m + 1) * P],
                )
                psum = psum_pool.tile([P, OUT], f32, tag="psum")
                for k in range(K_TILES):
                    if k < K_TILES - 1:
                        lhsT = xt_m[:, k * P:(k + 1) * P]
                    else:
                        lhsT = emb_t[:, m * P:(m + 1) * P]
                    for n in range(N_SPLIT):
                        nc.tensor.matmul(
                            psum[:, n * 512:(n + 1) * 512],
                            lhsT=lhsT,
                            rhs=wt_tiles[k][:, n * 512:(n + 1) * 512],
                            start=(k == 0),
                            stop=(k == K_TILES - 1),
                        )
                ot = out_pool.tile([P, OUT], f32, tag="ot")
                nc.vector.tensor_add(ot[:], psum[:], bias_t[:])
                nc.sync.dma_start(
                    out=out.ap()[m * P:(m + 1) * P, :], in_=ot[:])

    nc.compile()
    return nc


def _host_prep(inputs):
    x = np.ascontiguousarray(np.asarray(inputs["x"], dtype=np.float32))
    wv = np.asarray(inputs["weight_vals"], dtype=np.float32)
    wr = np.asarray(inputs["weight_rows"]).astype(np.int64)
    wc = np.asarray(inputs["weight_cols"]).astype(np.int64)
    bv = np.asarray(inputs["bias_vals"], dtype=np.float32)
    bi = np.asarray(inputs["bias_idx"]).astype(np.int64)
    e0v = np.asarray(inputs["embed0_vals"], dtype=np.float32)
    e0p = np.asarray(inputs["embed0_parents"]).astype(np.int64)
    e1v = np.asarray(inputs["embed1_vals"], dtype=np.float32)
    e1p = np.asarray(inputs["embed1_parents"]).astype(np.int64)

    # dense W^T [IN_TOT, OUT] (coalesce: duplicates sum)
    wt = np.bincount(wc * OUT + wr, weights=wv,
                     minlength=IN_TOT * OUT).reshape(IN_TOT, OUT)
    wt = np.ascontiguousarray(wt.astype(np.float32))

    b = np.bincount(bi, weights=bv, minlength=OUT).astype(np.float32)
    bias_bcast = np.ascontiguousarray(
        np.broadcast_to(b[None, :], (P, OUT)).astype(np.float32))

    # resolve embed parent chains to direct (row-in-x, multiplier) pairs
    q = np.empty(2 * N_EMBED, dtype=np.int32)
    a = np.empty(2 * N_EMBED, dtype=np.float32)
    q[:N_EMBED] = e0p
    a[:N_EMBED] = e0v
    for j in range(N_EMBED):
        p = int(e1p[j])
        if p < IN_BASE:
            q[N_EMBED + j] = p
            a[N_EMBED + j] = e1v[j]
        else:
            t = p - IN_BASE
            q[N_EMBED + j] = e0p[t]
            a[N_EMBED + j] = e1v[j] * e0v[t]

    xts = [np.ascontiguousarray(x[i * B_CORE:(i + 1) * B_CORE].T)
           for i in range(N_CORES)]
    return xts, wt, bias_bcast, q.reshape(P, 1), a.reshape(P, 1)


def kernel(**inputs) -> np.ndarray:
    from concourse.bass_utils import run_bass_kernel_spmd

    if "nc" not in _CACHED:
        _CACHED["nc"] = _build_nc()
    nc = _CACHED["nc"]

    xts, wt, bias_bcast, q, a = _host_prep(inputs)
    in_maps = [
        dict(xt=xts[i], wt=wt, bias=bias_bcast, emb_q=q, emb_a=a)
        for i in range(N_CORES)
    ]
    res = run_bass_kernel_spmd(nc, in_maps, core_ids=list(range(N_CORES)))
    out = np.concatenate([res.results[i]["out"] for i in range(N_CORES)],
                         axis=0)
    return np.ascontiguousarray(out.astype(np.float32))
